# revision 5
# baseline (speedup 1.0000x reference)
"""Trainium2 Bass kernel for the batched Kalman filter (nn_KalmanFilter).

Problem: x [B=8192, M=4, T=512]; F [8,8]; H [4,8]; Q [8,8]; R [4,4];
initial_std_dev [8].  Output: predicted observations out [B, M, T].

Key insight: the covariance recursion (P, S, K_t) is batch-independent
(identical initial covariance for every batch element), so the Kalman gains
K_t are precomputed on the host.  The remaining per-batch computation is a
LINEAR time-varying recurrence on an 8-dim state:

    mu_t  = A_t mu_{t-1} + K_t x_t,     A_t = (I - K_t H) F
    out_t = C mu_{t-1},                 C   = H F

which we evaluate as a chunked linear operator: T=512 is split into 16
chunks of 32 steps; within a chunk the map x->out is a dense 128x128 block
(Wd_j), the chunk states are propagated through an 8-dim bottleneck
(Gamma_j, L, G_j).  Everything becomes a handful of 128-contraction
matmuls per chunk -- ideal for the PE array.

Sharding: data-parallel over B across 8 NeuronCores (1024 batches/core).
The host pre-transposes x into [(chunk,(m,sigma)) = 2048, B_shard] layout so
the kernel needs no on-chip transposes; the kernel writes the output
transposed and the host transposes back.  Host work is O(B*M*T) numpy
reshapes; all heavy math runs on-device.
"""

import os
import numpy as np

# ---- problem constants (hardcoded; kernel.py must be self-contained) ----
B, M, NS, T = 8192, 4, 8, 512          # batches, measurements, states, steps
NCORES = 8
BS = B // NCORES                        # 1024 batches per core
CH = 32                                 # timesteps per chunk
NCH = T // CH                           # 16 chunks
MC = M * CH                             # 128 = chunk width in (m, sigma)
HB = 512                                # batch columns per compute phase
NH = BS // HB                           # 2 halves

_COMPILED = {}
LAST_EXEC_TIME_NS = None
LAST_RESULTS = None


# --------------------------------------------------------------------------
# Host-side precompute: Riccati recursion + chunked operator matrices.
# --------------------------------------------------------------------------
def _precompute(F, H, Q, R, initial_std_dev):
    """Build the chunked-operator constant matrices (float64 -> float32).

    Returns dict with:
      wdt  [128, 2048]  Wd_j^T  (within-chunk causal operator), chunk-major
      gamt [128, 2048]  Gamma_j^T zero-padded to [(i,n)=128] cols, chunk-major
      lt   [128, 128]   L^T     (chunk-state propagation, strictly blk-lower)
      gt   [8, 2048]    G_j^T   (chunk-start state -> chunk outputs)
    """
    F = np.asarray(F, np.float64)
    H = np.asarray(H, np.float64)
    Q = np.asarray(Q, np.float64)
    R = np.asarray(R, np.float64)
    s0 = np.asarray(initial_std_dev, np.float64)
    n, m = NS, M
    I = np.eye(n)
    C = H @ F
    P = np.diag(s0 ** 2)
    A = np.empty((T, n, n))
    Bm = np.empty((T, n, m))
    for t in range(T):
        Pp = F @ P @ F.T + Q
        S = H @ Pp @ H.T + R
        K = Pp @ H.T @ np.linalg.inv(S)
        ImKH = I - K @ H
        A[t] = ImKH @ F
        Bm[t] = K
        P = ImKH @ Pp @ ImKH.T + K @ R @ K.T

    Wd = np.zeros((NCH, MC, MC))
    G = np.zeros((NCH, MC, n))
    Gam = np.zeros((NCH, n, MC))
    Phic = np.zeros((NCH, n, n))
    rows_m = np.arange(m) * CH
    for j in range(NCH):
        t0 = j * CH
        phi = I.copy()
        for tau in range(CH):
            G[j][rows_m + tau, :] = C @ phi      # C Phi(t0+tau-1, t0-1)
            phi = A[t0 + tau] @ phi
        Phic[j] = phi
        for sigma in range(CH):
            X = Bm[t0 + sigma].copy()            # Phi(t0+sigma, t0+sigma) B
            for tau in range(sigma + 1, CH):
                Wd[j][rows_m + tau, sigma::CH] = C @ X
                X = A[t0 + tau] @ X
            Gam[j][:, sigma::CH] = X             # Phi(t0+CH-1, t0+sigma) B

    L = np.zeros((NCH * n, NCH * n))
    for j in range(NCH):
        X = I.copy()
        for i in range(j - 1, -1, -1):
            L[j * n:(j + 1) * n, i * n:(i + 1) * n] = X
            X = X @ Phic[i]

    wdt = np.concatenate([Wd[j].T for j in range(NCH)], axis=1)
    gam_big = np.zeros((NCH, MC, NCH * n))
    for j in range(NCH):
        gam_big[j][:, j * n:(j + 1) * n] = Gam[j].T
    gamt = np.concatenate(list(gam_big), axis=1)
    # G_j^T zero-padded to contract over the full MUt [128] (rhs base
    # partition must be 0): rows 8j:8j+8 of block j hold G_j^T.
    g_big = np.zeros((NCH, NCH * n, MC))
    for j in range(NCH):
        g_big[j][j * n:(j + 1) * n, :] = G[j].T
    gbigt = np.concatenate(list(g_big), axis=1)
    return {
        "wdt": np.ascontiguousarray(wdt, np.float32),
        "gamt": np.ascontiguousarray(gamt, np.float32),
        "lt": np.ascontiguousarray(L.T, np.float32),
        "gt": np.ascontiguousarray(gbigt, np.float32),
    }


# --------------------------------------------------------------------------
# Bass kernel builder
# --------------------------------------------------------------------------
def build_nc():
    import concourse.bacc as bacc
    import concourse.bass as bass
    import concourse.mybir as mybir
    import concourse.tile as tile

    ts = bass.ts
    dt = mybir.dt.float32

    nc = bacc.Bacc("TRN2", target_bir_lowering=False, debug=False,
                   num_devices=NCORES)
    xt_d = nc.dram_tensor("xt", [NCH * MC, BS], dt, kind="ExternalInput")
    wdt_d = nc.dram_tensor("wdt", [MC, NCH * MC], dt, kind="ExternalInput")
    gamt_d = nc.dram_tensor("gamt", [MC, NCH * MC], dt, kind="ExternalInput")
    lt_d = nc.dram_tensor("lt", [128, 128], dt, kind="ExternalInput")
    gt_d = nc.dram_tensor("gt", [MC, NCH * MC], dt, kind="ExternalInput")
    outt_d = nc.dram_tensor("outt", [NCH * MC, BS], dt, kind="ExternalOutput")

    with tile.TileContext(nc) as tc:
        with (
            tc.tile_pool(name="consts", bufs=1) as cpool,
            tc.tile_pool(name="xpool", bufs=1) as xpool,
            tc.tile_pool(name="opool", bufs=2) as opool,
            tc.tile_pool(name="stage", bufs=2) as spool,
            tc.tile_pool(name="st_ps", bufs=2, space="PSUM") as stps,
            tc.tile_pool(name="mu_ps", bufs=2, space="PSUM") as mups,
            tc.tile_pool(name="out_ps", bufs=3, space="PSUM") as outps,
        ):
            wdt = cpool.tile([MC, NCH * MC], dt)
            gamt = cpool.tile([MC, NCH * MC], dt)
            lt = cpool.tile([128, 128], dt)
            gt = cpool.tile([MC, NCH * MC], dt)
            nc.sync.dma_start(wdt[:], wdt_d[:])
            nc.sync.dma_start(gamt[:], gamt_d[:])
            nc.sync.dma_start(lt[:], lt_d[:])
            nc.sync.dma_start(gt[:], gt_d[:])

            # x, pre-transposed by host: row (128j + m*CH + sigma), col b
            xt = xpool.tile([128, NCH, BS], dt)
            xt_v = xt_d.rearrange("(j p) b -> p j b", p=128)
            for g in range(4):
                nc.sync.dma_start(xt[:, 4 * g:4 * g + 4, :],
                                  xt_v[:, 4 * g:4 * g + 4, :])

            outt_v = outt_d.rearrange("(j p) b -> p j b", p=128)

            for h in range(NH):
                hb = slice(h * HB, (h + 1) * HB)
                # ---- chunk-input states: St[(i,n)=128, b] ----
                st_ps = stps.tile([128, HB], dt, tag="st")
                for j in range(NCH):
                    nc.tensor.matmul(st_ps[:], gamt[:, ts(j, MC)],
                                     xt[:, j, hb],
                                     start=(j == 0), stop=(j == NCH - 1))
                st_sb = spool.tile([128, HB], dt, tag="st_sb")
                nc.vector.tensor_copy(st_sb[:], st_ps[:])
                # ---- chunk-start states: MUt = L^T.T @ St ----
                mu_ps = mups.tile([128, HB], dt, tag="mu")
                nc.tensor.matmul(mu_ps[:], lt[:], st_sb[:],
                                 start=True, stop=True)
                mu_sb = spool.tile([128, HB], dt, tag="mu_sb")
                nc.vector.tensor_copy(mu_sb[:], mu_ps[:])
                # ---- per-chunk outputs ----
                osb = opool.tile([128, NCH, HB], dt, tag="osb")
                for j in range(NCH):
                    ops = outps.tile([128, HB], dt, tag="ops")
                    nc.tensor.matmul(ops[:], gt[:, ts(j, MC)],
                                     mu_sb[:],
                                     start=True, stop=False)
                    nc.tensor.matmul(ops[:], wdt[:, ts(j, MC)],
                                     xt[:, j, hb],
                                     start=False, stop=True)
                    if j % 2 == 0:
                        nc.vector.tensor_copy(osb[:, j, :], ops[:])
                    else:
                        nc.scalar.copy(osb[:, j, :], ops[:])
                for g in range(4):
                    nc.sync.dma_start(outt_v[:, 4 * g:4 * g + 4, hb],
                                      osb[:, 4 * g:4 * g + 4, :])

    nc.compile()
    return nc


def _get_nc():
    if "nc" not in _COMPILED:
        _COMPILED["nc"] = build_nc()
    return _COMPILED["nc"]


# --------------------------------------------------------------------------
# Host data marshalling
# --------------------------------------------------------------------------
def _make_xt(x):
    """x [B, M, T] -> xT [NCH*MC, B] with row = 128j + m*CH + sigma."""
    xr = np.asarray(x, np.float32).reshape(B, M, NCH, CH)
    xt = xr.transpose(2, 1, 3, 0).reshape(NCH * MC, B)
    return np.ascontiguousarray(xt)


def _unmake_out(outt):
    """outT [NCH*MC, B] (row = 128j + m*CH + tau) -> out [B, M, T]."""
    o = outt.reshape(NCH, M, CH, B).transpose(3, 1, 0, 2)
    return np.ascontiguousarray(o.reshape(B, M, T))


def kernel(x, F, H, Q, R, initial_std_dev):
    global LAST_EXEC_TIME_NS, LAST_RESULTS
    from concourse import bass_utils

    consts = _precompute(F, H, Q, R, initial_std_dev)
    xt = _make_xt(x)
    nc = _get_nc()

    in_maps = []
    for k in range(NCORES):
        shard = np.ascontiguousarray(xt[:, k * BS:(k + 1) * BS])
        in_maps.append({"xt": shard, **consts})

    trace = bool(int(os.environ.get("KF_TRACE", "0")))
    res = bass_utils.run_bass_kernel_spmd(
        nc, in_maps, core_ids=list(range(NCORES)), trace=trace)
    LAST_RESULTS = res
    LAST_EXEC_TIME_NS = res.exec_time_ns

    outt = np.empty((NCH * MC, B), np.float32)
    for k in range(NCORES):
        outt[:, k * BS:(k + 1) * BS] = res.results[k]["outt"]
    return _unmake_out(outt)


# revision 6
# speedup vs baseline: 1.6402x; 1.6402x over previous
"""Trainium2 Bass kernel for the batched Kalman filter (nn_KalmanFilter).

Problem: x [B=8192, M=4, T=512]; F [8,8]; H [4,8]; Q [8,8]; R [4,4];
initial_std_dev [8].  Output: predicted observations out [B, M, T].

Key insight: the covariance recursion (P, S, K_t) is batch-independent
(identical initial covariance for every batch element), so the Kalman gains
K_t are precomputed on the host.  The remaining per-batch computation is a
LINEAR time-varying recurrence on an 8-dim state:

    mu_t  = A_t mu_{t-1} + K_t x_t,     A_t = (I - K_t H) F
    out_t = C mu_{t-1},                 C   = H F

which we evaluate as a chunked linear operator: T=512 is split into 16
chunks of 32 steps; within a chunk the map x->out is a dense 128x128 block
(Wd_j), the chunk states are propagated through an 8-dim bottleneck
(Gamma_j, L, G_j).  Everything becomes a handful of 128-contraction
matmuls per chunk -- ideal for the PE array.

Sharding: data-parallel over B across 8 NeuronCores (1024 batches/core).
The host pre-transposes x into [(chunk,(m,sigma)) = 2048, B_shard] layout so
the kernel needs no on-chip transposes; the kernel writes the output
transposed and the host transposes back.  Host work is O(B*M*T) numpy
reshapes; all heavy math runs on-device.
"""

import os
import numpy as np

# ---- problem constants (hardcoded; kernel.py must be self-contained) ----
B, M, NS, T = 8192, 4, 8, 512          # batches, measurements, states, steps
NCORES = 8
BS = B // NCORES                        # 1024 batches per core
CH = 32                                 # timesteps per chunk
NCH = T // CH                           # 16 chunks
MC = M * CH                             # 128 = chunk width in (m, sigma)
HB = 512                                # batch columns per compute phase
NH = BS // HB                           # 2 halves

_COMPILED = {}
LAST_EXEC_TIME_NS = None
LAST_RESULTS = None


# --------------------------------------------------------------------------
# Host-side precompute: Riccati recursion + chunked operator matrices.
# --------------------------------------------------------------------------
def _precompute(F, H, Q, R, initial_std_dev):
    """Build the chunked-operator constant matrices (float64 -> float32).

    Returns dict with:
      wdt  [128, 2048]  Wd_j^T  (within-chunk causal operator), chunk-major
      gamt [128, 2048]  Gamma_j^T zero-padded to [(i,n)=128] cols, chunk-major
      lt   [128, 128]   L^T     (chunk-state propagation, strictly blk-lower)
      gt   [8, 2048]    G_j^T   (chunk-start state -> chunk outputs)
    """
    F = np.asarray(F, np.float64)
    H = np.asarray(H, np.float64)
    Q = np.asarray(Q, np.float64)
    R = np.asarray(R, np.float64)
    s0 = np.asarray(initial_std_dev, np.float64)
    n, m = NS, M
    I = np.eye(n)
    C = H @ F
    P = np.diag(s0 ** 2)
    A = np.empty((T, n, n))
    Bm = np.empty((T, n, m))
    for t in range(T):
        Pp = F @ P @ F.T + Q
        S = H @ Pp @ H.T + R
        K = Pp @ H.T @ np.linalg.inv(S)
        ImKH = I - K @ H
        A[t] = ImKH @ F
        Bm[t] = K
        P = ImKH @ Pp @ ImKH.T + K @ R @ K.T

    Wd = np.zeros((NCH, MC, MC))
    G = np.zeros((NCH, MC, n))
    Gam = np.zeros((NCH, n, MC))
    Phic = np.zeros((NCH, n, n))
    rows_m = np.arange(m) * CH
    for j in range(NCH):
        t0 = j * CH
        phi = I.copy()
        for tau in range(CH):
            G[j][rows_m + tau, :] = C @ phi      # C Phi(t0+tau-1, t0-1)
            phi = A[t0 + tau] @ phi
        Phic[j] = phi
        for sigma in range(CH):
            X = Bm[t0 + sigma].copy()            # Phi(t0+sigma, t0+sigma) B
            for tau in range(sigma + 1, CH):
                Wd[j][rows_m + tau, sigma::CH] = C @ X
                X = A[t0 + tau] @ X
            Gam[j][:, sigma::CH] = X             # Phi(t0+CH-1, t0+sigma) B

    L = np.zeros((NCH * n, NCH * n))
    for j in range(NCH):
        X = I.copy()
        for i in range(j - 1, -1, -1):
            L[j * n:(j + 1) * n, i * n:(i + 1) * n] = X
            X = X @ Phic[i]

    wdt = np.concatenate([Wd[j].T for j in range(NCH)], axis=1)
    gam_big = np.zeros((NCH, MC, NCH * n))
    for j in range(NCH):
        gam_big[j][:, j * n:(j + 1) * n] = Gam[j].T
    gamt = np.concatenate(list(gam_big), axis=1)
    # G_j^T zero-padded to contract over the full MUt [128] (rhs base
    # partition must be 0): rows 8j:8j+8 of block j hold G_j^T.
    g_big = np.zeros((NCH, NCH * n, MC))
    for j in range(NCH):
        g_big[j][j * n:(j + 1) * n, :] = G[j].T
    gbigt = np.concatenate(list(g_big), axis=1)
    return {
        "wdt": np.ascontiguousarray(wdt, np.float32),
        "gamt": np.ascontiguousarray(gamt, np.float32),
        "lt": np.ascontiguousarray(L.T, np.float32),
        "gt": np.ascontiguousarray(gbigt, np.float32),
    }


# --------------------------------------------------------------------------
# Bass kernel builder
# --------------------------------------------------------------------------
def build_nc():
    import concourse.bacc as bacc
    import concourse.bass as bass
    import concourse.mybir as mybir
    import concourse.tile as tile

    ts = bass.ts
    # float32r: same 4-byte fp32 data, but the PE streams it at 1 cycle/row
    # for moving dims >= 256 (plain float32 lowers to 2 half-rate passes).
    dtc = mybir.dt.float32r
    dto = mybir.dt.float32

    nc = bacc.Bacc("TRN2", target_bir_lowering=False, debug=False,
                   num_devices=NCORES)
    xt_d = nc.dram_tensor("xt", [NCH * MC, BS], dtc, kind="ExternalInput")
    wdt_d = nc.dram_tensor("wdt", [MC, NCH * MC], dtc, kind="ExternalInput")
    gamt_d = nc.dram_tensor("gamt", [MC, NCH * MC], dtc, kind="ExternalInput")
    lt_d = nc.dram_tensor("lt", [128, 128], dtc, kind="ExternalInput")
    gt_d = nc.dram_tensor("gt", [MC, NCH * MC], dtc, kind="ExternalInput")
    outt_d = nc.dram_tensor("outt", [NCH * MC, BS], dto, kind="ExternalOutput")

    with tile.TileContext(nc) as tc:
        with (
            tc.tile_pool(name="consts", bufs=1) as cpool,
            tc.tile_pool(name="xpool", bufs=1) as xpool,
            tc.tile_pool(name="opool", bufs=2) as opool,
            tc.tile_pool(name="stage", bufs=2) as spool,
            tc.tile_pool(name="st_ps", bufs=2, space="PSUM") as stps,
            tc.tile_pool(name="mu_ps", bufs=2, space="PSUM") as mups,
            tc.tile_pool(name="out_ps", bufs=3, space="PSUM") as outps,
        ):
            wdt = cpool.tile([MC, NCH * MC], dtc)
            gamt = cpool.tile([MC, NCH * MC], dtc)
            lt = cpool.tile([128, 128], dtc)
            gt = cpool.tile([MC, NCH * MC], dtc)
            xt = xpool.tile([128, NCH, BS], dtc)
            xt_v = xt_d.rearrange("(j p) b -> p j b", p=128)

            # Two HWDGE rings: consts on the scalar ring, x / out on the
            # sync ring, ordered so the St phase can start ASAP.
            nc.scalar.dma_start(gamt[:], gamt_d[:])
            for h in range(NH):
                hb = slice(h * HB, (h + 1) * HB)
                for g in range(4):
                    nc.sync.dma_start(xt[:, 4 * g:4 * g + 4, hb],
                                      xt_v[:, 4 * g:4 * g + 4, hb])
                if h == 0:
                    nc.scalar.dma_start(wdt[:], wdt_d[:])
                    nc.scalar.dma_start(gt[:], gt_d[:])
                    nc.scalar.dma_start(lt[:], lt_d[:])

            outt_v = outt_d.rearrange("(j p) b -> p j b", p=128)

            for h in range(NH):
                hb = slice(h * HB, (h + 1) * HB)
                # ---- chunk-input states: St[(i,n)=128, b] ----
                st_ps = stps.tile([128, HB], dto, tag="st")
                for j in range(NCH):
                    nc.tensor.matmul(st_ps[:], gamt[:, ts(j, MC)],
                                     xt[:, j, hb],
                                     start=(j == 0), stop=(j == NCH - 1))
                st_sb = spool.tile([128, HB], dtc, tag="st_sb")
                nc.vector.tensor_copy(st_sb[:], st_ps[:])
                # ---- chunk-start states: MUt = L^T.T @ St ----
                mu_ps = mups.tile([128, HB], dto, tag="mu")
                nc.tensor.matmul(mu_ps[:], lt[:], st_sb[:],
                                 start=True, stop=True)
                mu_sb = spool.tile([128, HB], dtc, tag="mu_sb")
                nc.vector.tensor_copy(mu_sb[:], mu_ps[:])
                # ---- per-chunk outputs ----
                osb = opool.tile([128, NCH, HB], dto, tag="osb")
                for j in range(NCH):
                    ops = outps.tile([128, HB], dto, tag="ops")
                    nc.tensor.matmul(ops[:], gt[:, ts(j, MC)],
                                     mu_sb[:],
                                     start=True, stop=False)
                    nc.tensor.matmul(ops[:], wdt[:, ts(j, MC)],
                                     xt[:, j, hb],
                                     start=False, stop=True)
                    if j % 2 == 0:
                        nc.vector.tensor_copy(osb[:, j, :], ops[:])
                    else:
                        nc.scalar.copy(osb[:, j, :], ops[:])
                for g in range(4):
                    eng = nc.sync if g % 2 == 0 else nc.scalar
                    eng.dma_start(outt_v[:, 4 * g:4 * g + 4, hb],
                                  osb[:, 4 * g:4 * g + 4, :])

    nc.compile()
    return nc


def _get_nc():
    if "nc" not in _COMPILED:
        _COMPILED["nc"] = build_nc()
    return _COMPILED["nc"]


# --------------------------------------------------------------------------
# Host data marshalling
# --------------------------------------------------------------------------
def _make_xt(x):
    """x [B, M, T] -> xT [NCH*MC, B] with row = 128j + m*CH + sigma."""
    xr = np.asarray(x, np.float32).reshape(B, M, NCH, CH)
    xt = xr.transpose(2, 1, 3, 0).reshape(NCH * MC, B)
    return np.ascontiguousarray(xt)


def _unmake_out(outt):
    """outT [NCH*MC, B] (row = 128j + m*CH + tau) -> out [B, M, T]."""
    o = outt.reshape(NCH, M, CH, B).transpose(3, 1, 0, 2)
    return np.ascontiguousarray(o.reshape(B, M, T))


def kernel(x, F, H, Q, R, initial_std_dev):
    global LAST_EXEC_TIME_NS, LAST_RESULTS
    from concourse import bass_utils

    consts = _precompute(F, H, Q, R, initial_std_dev)
    xt = _make_xt(x)
    nc = _get_nc()

    in_maps = []
    for k in range(NCORES):
        shard = np.ascontiguousarray(xt[:, k * BS:(k + 1) * BS])
        in_maps.append({"xt": shard, **consts})

    trace = bool(int(os.environ.get("KF_TRACE", "0")))
    res = bass_utils.run_bass_kernel_spmd(
        nc, in_maps, core_ids=list(range(NCORES)), trace=trace)
    LAST_RESULTS = res
    LAST_EXEC_TIME_NS = res.exec_time_ns

    outt = np.empty((NCH * MC, B), np.float32)
    for k in range(NCORES):
        outt[:, k * BS:(k + 1) * BS] = res.results[k]["outt"]
    return _unmake_out(outt)
